# revision 67
# baseline (speedup 1.0000x reference)
"""Distributed Trainium2 Bass kernel for the AstraGNN message-passing wrapper.

Math (per iteration, reference):
    m      = relu([h_src, h_dst] @ W1 + b1) @ W2 + b2        (per edge)
    m      = m * edge_mask
    agg    = segment_sum(m, dst)
    h      = relu([h, agg] @ Wu + bu)
    logits = h @ Wo + bo                                      (returned for last iter)

Kernel reformulation:
    [h_src, h_dst] @ W1 = (h @ W1a)[src] + (h @ W1b)[dst]  with W1 = [W1a; W1b]
    segment_sum(relu(...) @ W2 + b2) = segment_sum(relu(...)) @ W2 + deg*b2
    => per-edge work reduces to: gather P[src], add Q[dst], relu, segment-sum,
       where the segment-sum and @W2 fuse into PSUM-accumulated matmuls over
       "slot rows" of a degree-staircase edge grid.

Distribution: nodes are sharded over 8 cores (dst-owner sharding).  Each core
computes P = h @ W1a for its shard; an AllGather replicates the P table to all
cores' DRAM; each core gathers P[src] for its own edges with dma_gather
(bf16, feature-major transpose mode).  Masked edges are dropped on the host.

Edge grid: per dst-tile (w=256/512 node columns), per table part, slot row c
holds the c-th in-edge of every dst in the tile.  Rows are cropped to a
staircase: within each tile dsts are sorted by part-degree so row c only
spans columns [0, width_c) (missing slots gather a pad row holding -1e4,
which relu kills).  The int16 gather-index limit (32767) is handled by
splitting the table into an A part (cores 0-4) and a B part (cores 5-7).

Gathers for a whole tile land in one staging SBUF buffer (double-buffered at
tile granularity) via large dma_gather calls on a SINGLE SWDGE queue, so the
previous tile's add/relu/matmul consumers overlap the next tile's gathers.
A single queue is mandatory: with >=2 queues, in-flight transposed gathers
complete out of order relative to the Tile framework's DMA-completion
semaphore thresholds and consumers read stale staging data (verified on HW;
per-call data itself is correct in isolation).  The P table is
double-buffered across iterations so the next AllGather never overwrites
rows an in-flight gather still reads.  Next-iteration P is computed per tile
right after each tile's h update, leaving only the collective itself on the
serial path between iterations.
"""

import sys

sys.path.insert(0, "/opt/trn_rl_repo")

import numpy as np
import ml_dtypes

import concourse.bass as bass
import concourse.mybir as mybir
from concourse import bacc
from concourse.tile import TileContext
from concourse.bass_utils import run_bass_kernel_spmd

BF16 = ml_dtypes.bfloat16
NCORES = 8
H = 128
ITERS = 3
HALF1 = 3072          # block-1 positions per core (= tiles 0..7)
CHUNK = 3584          # idxs per dma_gather (one call fills the 3584-desc ring)
SINGLE_PACKET = False  # multi-packet spreads one call across SDMA engines
NQUEUES = 1
SCRATCH = 57344       # descriptor-ring carveout: 3584-desc ring (= one call)
SNAP = 4              # staircase widths snap to multiples of this
NEG_BIG = -10000.0


# ---------------------------------------------------------------- host side

def _tile_widths(nloc):
    # heavier (low-position) tiles are narrower to bound the staging buffer
    sizes = [256, 256, 256, 256]
    left = nloc - sum(sizes)
    while left >= 512:
        sizes.append(512)
        left -= 512
    if left:
        sizes.append(left)
    return sizes


def _preprocess(x_nodes, edge_index, edge_mask):
    N = x_nodes.shape[0]
    src = np.asarray(edge_index[0], dtype=np.int64)
    dst = np.asarray(edge_index[1], dtype=np.int64)
    em = np.asarray(edge_mask, dtype=bool)
    src, dst = src[em], dst[em]

    nloc = ((N + NCORES - 1) // NCORES + 127) // 128 * 128  # per-core padded
    half1 = HALF1
    # --- shard assignment: sort by in-degree, round-robin over cores
    indeg = np.bincount(dst, minlength=N)
    order = np.argsort(-indeg, kind="stable")
    rank = np.empty(N, dtype=np.int64)
    rank[order] = np.arange(N)
    core = rank % NCORES
    pos0 = rank // NCORES  # per-core total-degree order

    # A/B split by POSITION block (not src core): block 1 = positions
    # [0, HALF1) = tiles 0..7.  The table is laid out block-major so each
    # block is one contiguous AllGather output, letting the two collectives
    # fire at different times.
    asplit = NCORES * half1  # rows in block 1 (A gathers use base row 0)
    is_a = pos0[src] < half1
    degA = np.bincount(dst[is_a], minlength=N)
    degB = np.bincount(dst[~is_a], minlength=N)

    tile_sizes = _tile_widths(nloc)
    tile_starts = np.concatenate([[0], np.cumsum(tile_sizes)])[:-1].astype(int)

    # --- within-core ordering: per BLOCK lexsort by (degA, degB) desc (so
    # nodes never cross the block boundary), then re-sort each tile's nodes
    # by (degB, degA) desc so BOTH staircases stay tight.
    pos = np.empty(N, dtype=np.int64)
    for c in range(NCORES):
        nodes = np.nonzero(core == c)[0]
        p0 = pos0[nodes]
        ordered = np.empty(nodes.size, dtype=np.int64)
        for blk in range(2):
            sel = np.nonzero((p0 < half1) == (blk == 0))[0]
            bn = nodes[sel]
            perm = np.lexsort((-degB[bn], -degA[bn]))
            if blk == 0:
                ordered[: bn.size] = bn[perm]
            else:
                ordered[half1 : half1 + bn.size] = bn[perm]
        out = []
        for t0, w in zip(tile_starts, tile_sizes):
            grp = ordered[t0 : t0 + w]
            p2 = np.lexsort((-degA[grp], -degB[grp]))
            out.append(grp[p2])
        ordered = np.concatenate(out)
        pos[ordered] = np.arange(ordered.size)
    label = core * nloc + pos
    src_l, dst_l = label[src], label[dst]

    # per-core per-pos part degrees
    cntA = np.zeros((NCORES, nloc), np.int64)
    cntB = np.zeros((NCORES, nloc), np.int64)
    cntA[core, pos] = degA
    cntB[core, pos] = degB

    # --- staircase widths per (tile, part, slot), uniform across cores
    # width = max over cores of (last position with deg>slot) + 1, snapped.
    # Each part's stream region is 128-aligned so gather calls (which have a
    # single table base) never cross the A/B boundary.
    # Rows are packed into gather calls so that NO row (hence no consumer op)
    # ever spans a call boundary; each call is 128-padded independently.
    tiles = []  # per tile: dict(t0, w, L, segs, calls, groups, bounds)
    stream_off = 0
    for t0, w in zip(tile_starts, tile_sizes):
        rows = []  # (part, width)
        for part, cnt in (("A", cntA), ("B", cntB)):
            seg = cnt[:, t0 : t0 + w]
            D = int(seg.max())
            for cc in range(D):
                m = seg > cc
                wc = 0
                for k in range(NCORES):
                    nz = np.nonzero(m[k])[0]
                    if nz.size:
                        wc = max(wc, int(nz[-1]) + 1)
                if wc == 0:
                    break
                rows.append((part, (wc + SNAP - 1) // SNAP * SNAP))
        # rows pack back-to-back; only the A/B part boundary is 128-aligned
        # (a call has a single table base).  Calls slice each part region
        # into CHUNK-sized gathers; rows may span calls (the single serial
        # SWDGE queue keeps completion in issue order).
        segs = []       # (part, stream_off_in_tile, width)
        groups = []     # [stream_off, nrows, width]
        off = 0
        bounds = {}
        for want in ("A", "B"):
            pstart = off
            for part, wc in rows:
                if part != want:
                    continue
                segs.append((part, off, wc))
                if groups and groups[-1][2] == wc and \
                   groups[-1][0] + groups[-1][1] * wc == off:
                    groups[-1][1] += 1
                else:
                    groups.append([off, 1, wc])
                off += wc
            off = (off + 127) // 128 * 128
            bounds[want] = (pstart, off)
        L = off
        calls = []
        for part in ("A", "B"):
            s, e = bounds[part]
            c0 = s
            while c0 < e:
                n = min(CHUNK, e - c0)
                calls.append((part, c0, n))
                c0 += n
        tiles.append(dict(t0=t0, w=w, L=L, segs=segs, calls=calls,
                          groups=groups, bounds=bounds,
                          stream_off=stream_off))
        stream_off += L
    tot_stream = stream_off
    totc = tot_stream // 16

    # --- per-core index streams
    nrows = NCORES * nloc + 2  # row0 = -BIG, rows 1..N*, last = -BIG
    pad_a = 0
    pad_b = (nrows - 1) - (asplit + 1)

    dcore = dst_l // nloc
    dloc = dst_l % nloc
    idx_streams = _build_streams(tiles, tot_stream, dcore, dloc, src_l, is_a,
                                 asplit, pad_a, pad_b, nloc)

    cnt_arr = np.zeros((NCORES, nloc), np.float32)
    cnt_arr[core, pos] = indeg  # total kept in-degree per node

    meta = dict(N=N, nloc=nloc, nrows=nrows, asplit=asplit, tiles=tiles,
                totc=totc, tot_stream=tot_stream)
    return meta, idx_streams, cnt_arr, label


def _table_row(label, nloc, asplit):
    """Block-major table row of a node label (block1 then block2)."""
    cr = label // nloc
    pp = label % nloc
    return np.where(pp < HALF1, 1 + cr * HALF1 + pp,
                    1 + asplit + cr * (nloc - HALF1) + (pp - HALF1))


def _build_streams(tiles, tot_stream, dcore, dloc, src_l, is_a,
                   asplit, pad_a, pad_b, nloc):
    """Vectorized per-core index-stream builder."""
    streams = []
    for c in range(NCORES):
        m = dcore == c
        dl, sl, ia = dloc[m], src_l[m], is_a[m]
        stream = np.empty(tot_stream, np.int16)
        # fill each part region with its pad value
        for T in tiles:
            base = T["stream_off"]
            for part in ("A", "B"):
                s, e = T["bounds"][part]
                stream[base + s : base + e] = pad_a if part == "A" else pad_b
        for part in ("A", "B"):
            sel = ia if part == "A" else ~ia
            d_p, s_p = dl[sel], sl[sel]
            o = np.lexsort((s_p, d_p))
            d_p, s_p = d_p[o], s_p[o]
            first = np.concatenate([[True], d_p[1:] != d_p[:-1]])
            gstart = np.nonzero(first)[0]
            slot = np.arange(d_p.size) - np.repeat(
                gstart, np.diff(np.concatenate([gstart, [d_p.size]])))
            row = _table_row(s_p, nloc, asplit)
            if part == "A":
                val = row.astype(np.int16)
            else:
                val = (row - (asplit + 1)).astype(np.int16)
            # per tile: map (slot, dcol) -> stream position
            for T in tiles:
                t0, w, base = T["t0"], T["w"], T["stream_off"]
                # slot-row offsets for this part
                offs = {}
                for part2, soff, wc in T["segs"]:
                    if part2 == part:
                        offs[len(offs)] = (soff, wc)
                if not offs:
                    continue
                mm = (d_p >= t0) & (d_p < t0 + w)
                dmm = d_p[mm] - t0
                vmm = val[mm]
                smm = slot[mm]
                offarr = np.array([offs[k][0] for k in range(len(offs))],
                                  np.int64)
                posn = base + offarr[smm] + dmm
                stream[posn] = vmm
        streams.append(np.tile(stream.reshape(tot_stream // 16, 16).T,
                               (8, 1)))  # [128, totc]
    return streams


# ------------------------------------------------------------- device side

_PROGRAM_CACHE = {}


def _build_program(meta):
    key = (meta["N"], meta["tot_stream"],
           tuple((T["t0"], T["w"], T["L"]) for T in meta["tiles"]))
    if key in _PROGRAM_CACHE:
        return _PROGRAM_CACHE[key]

    nloc = meta["nloc"]
    nrows = meta["nrows"]
    asplit = meta["asplit"]
    tiles = meta["tiles"]
    totc = meta["totc"]
    nblk = nloc // 128
    bf = mybir.dt.bfloat16
    f32 = mybir.dt.float32
    AF = mybir.ActivationFunctionType

    nc = bacc.Bacc("TRN2", target_bir_lowering=False, debug=False,
                   num_devices=NCORES, num_swdge_queues=max(NQUEUES, 1),
                   dynamic_dma_scratch_size=SCRATCH)

    # external I/O
    hT0_e = nc.dram_tensor("hT0", [128, nloc], bf, kind="ExternalInput")
    idx_e = nc.dram_tensor("idx", [128, totc], mybir.dt.int16,
                           kind="ExternalInput")
    cnt_e = nc.dram_tensor("cnt", [1, nloc], bf, kind="ExternalInput")
    w1a_e = nc.dram_tensor("W1a", [128, 128], bf, kind="ExternalInput")
    w1b_e = nc.dram_tensor("W1b", [128, 128], bf, kind="ExternalInput")
    w2_e = nc.dram_tensor("W2", [128, 128], bf, kind="ExternalInput")
    wut_e = nc.dram_tensor("Wut", [128, 128], bf, kind="ExternalInput")
    wub_e = nc.dram_tensor("Wub", [128, 128], bf, kind="ExternalInput")
    wo_e = nc.dram_tensor("Wo", [128, 2], bf, kind="ExternalInput")
    b1_e = nc.dram_tensor("b1", [128, 1], f32, kind="ExternalInput")
    bu_e = nc.dram_tensor("bu", [1, 128], bf, kind="ExternalInput")
    b2h_e = nc.dram_tensor("b2h", [1, 128], bf, kind="ExternalInput")
    bo_e = nc.dram_tensor("bo", [1, 2], bf, kind="ExternalInput")
    out_e = nc.dram_tensor("out", [nloc, 2], f32, kind="ExternalOutput")

    # internal DRAM — per position-block tables (separate tensors so A-part
    # gathers depend only on CC1, not CC2), double-buffered across
    # iterations so the next AllGather never overwrites rows an in-flight
    # gather still reads.
    a_rows = 1 + asplit                      # pad row 0 + block-1 rows
    b_rows = NCORES * (nloc - HALF1) + 1     # block-2 rows + pad last row
    tablesA = [nc.dram_tensor(f"ptA{i}", [a_rows, 128], bf,
                              addr_space="Shared") for i in range(2)]
    tablesB = [nc.dram_tensor(f"ptB{i}", [b_rows, 128], bf,
                              addr_space="Shared") for i in range(2)]
    ag_in = nc.dram_tensor("ag_in", [nloc, 128], bf)

    max_even = max(T["L"] for i, T in enumerate(tiles) if i % 2 == 0)
    max_odd = max(T["L"] for i, T in enumerate(tiles) if i % 2 == 1)

    with TileContext(nc) as tc:
        with (
            tc.tile_pool(name="res", bufs=1) as res,
            tc.tile_pool(name="st0", bufs=2) as st0,
            tc.tile_pool(name="st1", bufs=2) as st1,
            tc.tile_pool(name="spool", bufs=2) as spool,
            tc.tile_pool(name="pe_psum", bufs=2, space="PSUM") as pe_psum,
            tc.tile_pool(name="pa_psum", bufs=2, space="PSUM") as pa_psum,
            tc.tile_pool(name="pu_psum", bufs=2, space="PSUM") as pu_psum,
        ):
            # ---- residents
            idx_sb = res.tile([128, totc], mybir.dt.int16, tag="idx")
            hT = [res.tile([128, nloc], bf, tag=f"hT{i}", name=f"hT{i}")
                  for i in range(2)]
            Q_sb = res.tile([128, nloc], bf, tag="Q")
            P_st = res.tile([128, nloc], bf, tag="Pst")
            cnt_sb = res.tile([1, nloc], bf, tag="cnt")
            w1a = res.tile([128, 128], bf, tag="w1a")
            w1b = res.tile([128, 128], bf, tag="w1b")
            w2 = res.tile([128, 128], bf, tag="w2")
            wut = res.tile([128, 128], bf, tag="wut")
            wub = res.tile([128, 128], bf, tag="wub")
            wo = res.tile([128, 2], bf, tag="wo")
            b1 = res.tile([128, 1], f32, tag="b1")
            bu = res.tile([1, 128], bf, tag="bu")
            b2h = res.tile([1, 128], bf, tag="b2h")
            bo = res.tile([1, 2], bf, tag="bo")
            ones = res.tile([1, 512], bf, tag="ones")
            negr = res.tile([1, 128], bf, tag="negr")
            lst = res.tile([128, nblk * 2], f32, tag="lst")


            for t, e in [(idx_sb, idx_e), (hT[0], hT0_e), (cnt_sb, cnt_e),
                         (w1a, w1a_e), (w1b, w1b_e), (w2, w2_e), (wut, wut_e),
                         (wub, wub_e), (wo, wo_e), (b1, b1_e), (bu, bu_e),
                         (b2h, b2h_e), (bo, bo_e)]:
                nc.sync.dma_start(out=t[:], in_=e.ap())
            nc.vector.memset(ones[:], 1.0)
            nc.vector.memset(negr[:], NEG_BIG)
            for tA in tablesA:
                nc.sync.dma_start(out=tA.ap()[0:1, :], in_=negr[:])
            for tB in tablesB:
                nc.sync.dma_start(out=tB.ap()[b_rows - 1 : b_rows, :],
                                  in_=negr[:])

            ag_in_3d = ag_in.ap().rearrange("(b p) m -> p b m", p=128)
            out_3d = out_e.ap().rearrange("(b p) o -> p b o", p=128)
            qrot = [0]
            stage_pools = [st0, st1]
            stage_sizes = [max_even, max_odd]

            # ---- prologue: P for iteration 0 (later iterations compute P
            # per-tile inside phase B, right after each tile's h update)
            for b in range(nblk):
                ps = pa_psum.tile([128, 128], f32, tag="psA")
                nc.tensor.matmul(out=ps[:],
                                 lhsT=hT[0][:, b * 128 : (b + 1) * 128],
                                 rhs=w1a[:], start=True, stop=True)
                nc.scalar.activation(out=P_st[:, b * 128 : (b + 1) * 128],
                                     in_=ps[:], func=AF.Copy)
            nc.sync.dma_start(
                out=ag_in_3d,
                in_=P_st[:].rearrange("p (b m) -> p b m", m=128),
            )

            def emit_cc(it2, blk):
                # AllGather one position block of iteration it2's tables.
                if blk == 0:
                    nc.gpsimd.collective_compute(
                        "AllGather", mybir.AluOpType.bypass,
                        replica_groups=[list(range(NCORES))],
                        ins=[ag_in.ap()[0:HALF1, :].opt()],
                        outs=[tablesA[it2 % 2].ap()[1 : 1 + asplit, :].opt()],
                    )
                else:
                    nc.gpsimd.collective_compute(
                        "AllGather", mybir.AluOpType.bypass,
                        replica_groups=[list(range(NCORES))],
                        ins=[ag_in.ap()[HALF1:nloc, :].opt()],
                        outs=[tablesB[it2 % 2].ap()[0 : b_rows - 1, :].opt()],
                    )

            # iteration 0's collectives (nothing to hide them under)
            emit_cc(0, 0)
            emit_cc(0, 1)

            for it in range(ITERS):
                tbl_a = tablesA[it % 2].ap()
                tbl_b = tablesB[it % 2].ap()
                h = hT[it % 2]
                hn = hT[(it + 1) % 2]

                for q0 in range(0, nloc, 512):
                    qw = min(512, nloc - q0)
                    ps = pa_psum.tile([128, 512], f32, tag="psA")
                    nc.tensor.matmul(out=ps[:, :qw], lhsT=w1b[:],
                                     rhs=h[:, q0 : q0 + qw],
                                     start=True, stop=True)
                    nc.scalar.activation(out=Q_sb[:, q0 : q0 + qw],
                                         in_=ps[:, :qw],
                                         func=AF.Identity, bias=b1[:])

                # ---- phase B: staged gathers + staircase accumulation
                for ti, T in enumerate(tiles):
                    t0, w, L = T["t0"], T["w"], T["L"]
                    pool_i = ti % 2
                    sp = stage_pools[pool_i]
                    S = sp.tile([128, stage_sizes[pool_i]], bf, tag="S")
                    base_c = T["stream_off"] // 16
                    for (part, coff, n) in T["calls"]:
                        gv = S[:, coff : coff + n].rearrange(
                            "p (a n) -> p a n", a=1)
                        nc.gpsimd.dma_gather(
                            gv,
                            tbl_a if part == "A" else tbl_b,
                            idx_sb[:, base_c + coff // 16 :
                                   base_c + (coff + n) // 16],
                            num_idxs=n,
                            num_idxs_reg=n,
                            elem_size=128,
                            transpose=True,
                            queue_num=qrot[0],
                            single_packet=SINGLE_PACKET,
                        )
                        qrot[0] = (qrot[0] + 1) % NQUEUES

                    # add Q + relu per equal-width group
                    for goff, nb, wc in T["groups"]:
                        g3 = S[:, goff : goff + nb * wc].rearrange(
                            "p (b d) -> p b d", d=wc)
                        qb = Q_sb[:, t0 : t0 + wc].unsqueeze(1).to_broadcast(
                            [128, nb, wc])
                        nc.vector.tensor_tensor(out=g3, in0=g3, in1=qb,
                                                op=mybir.AluOpType.add)
                        nc.scalar.activation(out=g3, in_=g3, func=AF.Relu)

                    # staircase matmul accumulation into psE
                    psE = pe_psum.tile([128, 512], f32, tag="psE")
                    nc.tensor.matmul(out=psE[:, :w], lhsT=b2h[:],
                                     rhs=cnt_sb[:, t0 : t0 + w],
                                     start=True, stop=False)
                    for part, soff, wc in T["segs"]:
                        nc.tensor.matmul(
                            out=psE[:, :wc], lhsT=w2[:],
                            rhs=S[:, soff : soff + wc],
                            start=False, stop=False)
                    nc.tensor.matmul(out=psE[:, :w], lhsT=b2h[:],
                                     rhs=cnt_sb[:, t0 : t0 + w],
                                     start=False, stop=True)

                    agg = spool.tile([128, 512], bf, tag="agg")
                    nc.scalar.activation(out=agg[:, :w], in_=psE[:, :w],
                                         func=AF.Copy)
                    psU = pu_psum.tile([128, 512], f32, tag="psU")
                    nc.tensor.matmul(out=psU[:, :w], lhsT=wut[:],
                                     rhs=h[:, t0 : t0 + w],
                                     start=True, stop=False)
                    nc.tensor.matmul(out=psU[:, :w], lhsT=wub[:],
                                     rhs=agg[:, :w], start=False, stop=False)
                    nc.tensor.matmul(out=psU[:, :w], lhsT=bu[:],
                                     rhs=ones[:, :w], start=False, stop=True)
                    nc.scalar.activation(out=hn[:, t0 : t0 + w],
                                         in_=psU[:, :w], func=AF.Relu)

                    if it < ITERS - 1:
                        # next iteration's P for this tile's columns, so only
                        # the collective itself stays on the serial path
                        for b in range(t0 // 128, (t0 + w) // 128):
                            ps = pa_psum.tile([128, 128], f32, tag="psA")
                            nc.tensor.matmul(
                                out=ps[:],
                                lhsT=hn[:, b * 128 : (b + 1) * 128],
                                rhs=w1a[:], start=True, stop=True)
                            nc.scalar.activation(
                                out=P_st[:, b * 128 : (b + 1) * 128],
                                in_=ps[:], func=AF.Copy)
                        nc.sync.dma_start(
                            out=ag_in_3d[:, t0 // 128 : (t0 + w) // 128, :],
                            in_=P_st[:, t0 : t0 + w].rearrange(
                                "p (b m) -> p b m", m=128),
                        )
                        # block-1 positions complete after tile 7; emit CC1
                        # two tiles later so its input dep is already met
                        # and the gpsimd queue never stalls waiting for it.
                        # It hides under tiles 10..14's gathers; CC2 fires
                        # at the end and hides under the next iteration's
                        # A-part gathers (which depend only on tablesA).
                        if t0 + w == HALF1 + 1024:
                            emit_cc(it + 1, 0)
                        if ti == len(tiles) - 1:
                            emit_cc(it + 1, 1)

            # ---- output head on final h
            hfin = hT[ITERS % 2]
            for b in range(nblk):
                ps = pa_psum.tile([128, 2], f32, tag="psL")
                nc.tensor.matmul(out=ps[:],
                                 lhsT=hfin[:, b * 128 : (b + 1) * 128],
                                 rhs=wo[:], start=True, stop=False)
                nc.tensor.matmul(out=ps[:], lhsT=ones[:, :128], rhs=bo[:],
                                 start=False, stop=True)
                nc.vector.tensor_copy(out=lst[:, b * 2 : b * 2 + 2], in_=ps[:])
            nc.sync.dma_start(out=out_3d,
                              in_=lst[:].rearrange("p (b o) -> p b o", o=2))

    nc.compile()
    _PROGRAM_CACHE[key] = nc
    return nc


# --------------------------------------------------------------- interface

def kernel(x_nodes, edge_index, edge_attr, node_mask, edge_mask,
           W1, b1, W2, b2, Wu, bu, Wo, bo):
    x_nodes = np.asarray(x_nodes, dtype=np.float32)
    meta, idx_streams, cnt_arr, label = _preprocess(x_nodes, edge_index,
                                                    edge_mask)
    nloc = meta["nloc"]
    N = meta["N"]

    nc = _build_program(meta)

    W1 = np.asarray(W1, np.float32)
    Wu = np.asarray(Wu, np.float32)
    shared = dict(
        W1a=W1[:H].astype(BF16), W1b=W1[H:].astype(BF16),
        W2=np.asarray(W2, np.float32).astype(BF16),
        Wut=Wu[:H].astype(BF16), Wub=Wu[H:].astype(BF16),
        Wo=np.asarray(Wo, np.float32).astype(BF16),
        b1=np.asarray(b1, np.float32).reshape(128, 1),
        bu=np.asarray(bu, np.float32).reshape(1, 128).astype(BF16),
        b2h=(np.asarray(b2, np.float32).reshape(1, 128) / 2).astype(BF16),
        bo=np.asarray(bo, np.float32).reshape(1, 2).astype(BF16),
    )

    in_maps = []
    for c in range(NCORES):
        hT0 = np.zeros((128, nloc), BF16)
        sel = (label // nloc) == c
        hT0[:, label[sel] % nloc] = x_nodes[sel].T.astype(BF16)
        in_maps.append(dict(
            hT0=hT0,
            idx=idx_streams[c],
            cnt=cnt_arr[c].reshape(1, nloc).astype(BF16),
            **shared,
        ))

    global _last_in_maps
    _last_in_maps = in_maps
    import os
    trace = bool(os.environ.get("KERNEL_TRACE"))
    res = run_bass_kernel_spmd(nc, in_maps, core_ids=list(range(NCORES)),
                               trace=trace)
    if trace:
        print(f"HW exec time: {res.exec_time_ns} ns")
    full = np.concatenate([r["out"] for r in res.results], axis=0)
    by_label = full.reshape(NCORES * nloc, 2)
    return by_label[label].astype(np.float32)
